# revision 70
# baseline (speedup 1.0000x reference)
"""Trainium2 Bass kernel for nn_GAT_n2v_mean (3-layer edge-featured GAT + mean-pool + MLP).

Strategy (hardcoded): partition edges by dst owner across 8 cores (6250 nodes
each, dst-sorted, 127-node blocks + trash slot, per-block exact tile counts).
Self-loops are appended as ordinary edges on the host (edge_attr = per-dst
mean, PyG fill_value='mean'). All input-only quantities (edge means, al_e
logit terms) are precomputed on the host. Per layer: phase A computes an fp8
node table [xs | al_s] plus a bf16 al_d table (BN scale folded into the
weights, al_s/al_d projections folded in as extra matmul columns), the fp8
table is AllGathered, then a fully-unrolled per-block attention loop issues
one indirect row-gather per 128-edge tile (Pool engine kept DMA-only),
builds per-tile one-hot matrices with 4x-mode tensor_scalar ops, expands
al_d[dst] per edge via PE-transposed one-hots and tiny matmuls, computes
softmax weights without any max shift (logits are bounded; the shift
cancels), and aggregates via PSUM matmuls. The channel layout is interleaved
(c-major, h-minor); logits/exp stay fp32->bf16. Cross-block pipelining
relies on multi-buffered tiles, record prefetch two blocks ahead, a widened
SWDGE descriptor ring, and row-major hidden-state stores (transposes moved
to phase A, off the per-block critical path).
"""

import numpy as np
import ml_dtypes

BF = ml_dtypes.bfloat16

_PATCHED = False


def _patch_walrus():
    """Enable per-partition vector dynamic offsets in walrus codegen
    (needed for the indirect row gathers; off by default in this path)."""
    global _PATCHED
    if _PATCHED:
        return
    import concourse.bass_utils as _bu
    _orig = _bu.run_command

    def _patched(argv, **kw):
        if any("codegen" in str(a) for a in argv):
            argv = list(argv)
            i = argv.index("-i")
            argv[i:i] = ["--dge-levels=vector_dynamic_offsets"]
        return _orig(argv, **kw)

    _bu.run_command = _patched
    _PATCHED = True


# ---------------------------------------------------------------- host config
N, E, G, D = 50000, 800000, 64, 8
NPD = N // D          # nodes per device
BLK = 127             # real node slots per block (slot 127 = trash)
NB = (NPD + BLK - 1) // BLK
R = NB * 128          # padded local rows in the node table
EPS = 1e-5
BNC = float(1.0 / np.sqrt(1.0 + EPS))
DIMS = [(32, 4, 64), (256, 4, 128), (512, 4, 64)]

_CACHE = {}


def _interleave_cols(H, C):
    """new col (c*H + h) <- old col (h*C + c); returns old index per new col."""
    newc = np.arange(H * C)
    c, h = newc // H, newc % H
    return h * C + c


def _prep(inputs):
    """Host-side sharding/layout prep (numpy only). Returns (in_maps, T)."""
    src_g = np.asarray(inputs["edge_index"][0], dtype=np.int64)
    dst_g = np.asarray(inputs["edge_index"][1], dtype=np.int64)
    ef = np.asarray(inputs["edge_feature"], dtype=np.float32)
    batch = np.asarray(inputs["batch"], dtype=np.int64)
    x = np.asarray(inputs["x"], dtype=np.float32)

    # self-loop edge features: per-dst mean of incoming edge features
    deg = np.bincount(dst_g, minlength=N).astype(np.float32)
    emean = np.zeros((N, 6), np.float32)
    np.add.at(emean, dst_g, ef)
    emean /= np.maximum(deg, 1.0)[:, None]

    src_a = np.concatenate([src_g, np.arange(N, dtype=np.int64)])
    dst_a = np.concatenate([dst_g, np.arange(N, dtype=np.int64)])
    ea_a = np.concatenate([ef, emean], axis=0)

    # per-layer host-computable logit projections Ae[f, h] and weight folds
    Ae_l, Wcat_l, b2_l = [], [], []
    prev_n2o = None
    for li, (fin, H, C) in enumerate(DIMS, 1):
        HC = H * C
        W = np.asarray(inputs[f"W{li}"], np.float32)
        a_s = np.asarray(inputs[f"as{li}"], np.float32)
        a_d = np.asarray(inputs[f"ad{li}"], np.float32)
        a_e = np.asarray(inputs[f"ae{li}"], np.float32)
        g_ = np.asarray(inputs[f"g{li}"], np.float32)
        b_ = np.asarray(inputs[f"b{li}"], np.float32)
        bb_ = np.asarray(inputs[f"bb{li}"], np.float32)
        We = np.asarray(inputs[f"We{li}"], np.float32)

        Ae = np.zeros((6, H), np.float32)
        WA = np.zeros((fin, 8), np.float32)
        for h in range(H):
            Ae[:, h] = We[:, h * C:(h + 1) * C] @ a_e[h]
            WA[:, h] = W[:, h * C:(h + 1) * C] @ a_s[h]
            WA[:, 4 + h] = W[:, h * C:(h + 1) * C] @ a_d[h]
        Ae_l.append(Ae)

        gc = g_ * BNC
        n2o = _interleave_cols(H, C)
        Wcat = np.concatenate([(W * gc[None, :])[:, n2o], WA], axis=1)
        if prev_n2o is not None:
            Wcat = Wcat[prev_n2o, :]
        Wcat_l.append(Wcat.astype(BF))
        b2_l.append((b_ * gc + bb_)[n2o].astype(BF))
        prev_n2o = n2o

    per_dev = []
    owner = dst_a // NPD
    for d in range(D):
        m = owner == d
        s, t = src_a[m], dst_a[m]
        ea = ea_a[m]
        loc = t - d * NPD
        blk = loc // BLK
        rel = loc % BLK
        order = np.argsort(blk, kind="stable")
        s, ea, blk, rel = s[order], ea[order], blk[order], rel[order]
        cnt = np.bincount(blk, minlength=NB)
        per_dev.append((s, ea, blk, rel, cnt))

    # dense tile-major slotting: slot k = (t = k // 128, p = k % 128)
    # per-block tile counts: max over devices (SPMD program is shared)
    cnts = np.stack([per_dev[d][4] for d in range(D)])      # [D, NB]
    Tb = tuple(int(t) for t in np.ceil(cnts.max(axis=0) / 128).astype(int))
    T = max(Tb)

    in_maps = []
    for d in range(D):
        s, ea, blk, rel, cnt = per_dev[d]
        idx_s = np.zeros((NB, T * 128), np.int32)     # global row in xf
        relm = np.full((NB, T * 128), 127.0, np.float32)
        alem = np.zeros((NB, T * 128, 12), np.float32)
        off = np.concatenate([[0], np.cumsum(cnt)])
        for bi in range(NB):
            e0, e1 = off[bi], off[bi + 1]
            k = e1 - e0
            ss = s[e0:e1]
            idx_s[bi, :k] = ((ss // NPD) * R + (ss % NPD)).astype(np.int32)
            relm[bi, :k] = rel[e0:e1].astype(np.float32)
            alem[bi, :k, :] = np.stack(
                [ea[e0:e1] @ Ae_l[li] for li in range(3)],
                axis=1).reshape(-1, 12)
        tp = lambda a: a.reshape(NB, T, 128).transpose(0, 2, 1)
        bb = np.full((NB, 128, 1), -1.0, np.float32)
        for bi in range(NB):
            lo = bi * BLK
            n = min(BLK, NPD - lo)
            if n > 0:
                bb[bi, :n, 0] = batch[d * NPD + lo: d * NPD + lo + n]
        # ale cols layer-major: l*4T + t*4 + h
        aleT = alem.reshape(NB, T, 128, 12).transpose(0, 2, 3, 1)
        aleT = aleT.reshape(NB, 128, 3, 4, T).transpose(0, 1, 2, 4, 3)
        aleT = aleT.reshape(NB, 128, 12 * T)
        rec = np.concatenate(
            [tp(idx_s), tp(relm.view(np.int32)), bb.view(np.int32),
             aleT.view(np.int32)], axis=2
        ).reshape(NB * 128, 14 * T + 1).copy()

        xT = np.zeros((32, R), BF)
        xT[:, :NPD] = x[d * NPD:(d + 1) * NPD].T.astype(BF)

        im = {
            "xT": xT,
            "rec": rec,
            "io128": np.broadcast_to(np.arange(128, dtype=np.float32),
                                     (128, 128)).astype(BF).copy(),
            "io64": np.broadcast_to(np.arange(64, dtype=np.float32),
                                    (128, 64)).astype(BF).copy(),
            "ident": np.eye(128, dtype=np.float32).astype(BF),
            "identf": np.eye(64, dtype=np.float32),
        }
        for li in range(1, 4):
            im[f"Wcat{li}"] = Wcat_l[li - 1]
            im[f"b2r{li}"] = np.broadcast_to(
                b2_l[li - 1], (128, len(b2_l[li - 1]))).copy()
        n2o3 = _interleave_cols(4, 64)
        im["Wf1"] = np.asarray(inputs["Wf1"], np.float32)[n2o3, :].copy()
        im["Wf2"] = np.asarray(inputs["Wf2"], np.float32)
        im["bf1r"] = np.broadcast_to(np.asarray(inputs["bf1"], np.float32),
                                     (64, 32)).copy()
        im["gfr"] = np.broadcast_to(np.asarray(inputs["gf"], np.float32),
                                    (64, 32)).copy()
        im["bbfr"] = np.broadcast_to(np.asarray(inputs["bbf"], np.float32),
                                     (64, 32)).copy()
        im["bf2r"] = np.broadcast_to(np.asarray(inputs["bf2"], np.float32),
                                     (64, 2)).copy()
        in_maps.append(im)
    return in_maps, (T, Tb)


def nk2(li):
    return DIMS[li - 1][1] * DIMS[li - 1][2] // 128


# ---------------------------------------------------------------- device prog
def _build(TT_):
    T, Tb = TT_
    import concourse.bass as bass
    import concourse.bacc as bacc
    import concourse.mybir as mybir
    import concourse.tile as tile
    from contextlib import ExitStack

    f32 = mybir.dt.float32
    bf16 = mybir.dt.bfloat16
    f8 = mybir.dt.float8e4
    i32 = mybir.dt.int32
    AO = mybir.AluOpType
    AF = mybir.ActivationFunctionType
    RG = [list(range(D))]
    RECW = 14 * T + 1

    nc = bacc.Bacc(None, target_bir_lowering=False, debug=True,
                   dynamic_dma_scratch_size=49152)

    # ---- I/O
    inp = {}
    def di(name, shape, dt=f32):
        inp[name] = nc.declare_dram_parameter(name, list(shape), dt,
                                              isOutput=False)
        return inp[name]

    di("xT", (32, R), bf16)
    di("rec", (NB * 128, RECW), i32)
    di("io128", (128, 128), bf16)
    di("io64", (128, 64), bf16)
    di("ident", (128, 128), bf16)
    di("identf", (64, 64))
    for li, (fin, H, C) in enumerate(DIMS, 1):
        HC = H * C
        di(f"Wcat{li}", (fin, HC + 8), bf16)
        di(f"b2r{li}", (128, HC), bf16)
    di("Wf1", (256, 32)); di("Wf2", (32, 2))
    di("bf1r", (64, 32)); di("gfr", (64, 32)); di("bbfr", (64, 32))
    di("bf2r", (64, 2))
    out_d = nc.declare_dram_parameter("out", [64, 2], f32, isOutput=True)

    # ---- internal DRAM
    Ws = [DIMS[i][1] * DIMS[i][2] for i in range(3)]
    xe_d = [nc.dram_tensor(f"xe{l}", [R, Ws[l - 1] + 4], f8)
            for l in (1, 2, 3)]
    xf_d = [nc.dram_tensor(f"xf{l}", [D * R, Ws[l - 1] + 4], f8,
                           addr_space="Shared") for l in (1, 2, 3)]
    ald_d = [nc.dram_tensor(f"ald{l}", [R, 4], bf16) for l in (1, 2, 3)]
    hT_d = [None,
            nc.dram_tensor("hT1", [R, 256], bf16),
            nc.dram_tensor("hT2", [R, 512], bf16)]
    pool_i = nc.dram_tensor("pool_i", [64, 257], f32)
    pool_o = nc.dram_tensor("pool_o", [64, 257], f32, addr_space="Shared")

    with ExitStack() as ctx:
        tc = ctx.enter_context(tile.TileContext(nc))
        consts = ctx.enter_context(tc.tile_pool(name="consts", bufs=1))
        lay = ctx.enter_context(tc.tile_pool(name="lay", bufs=1))
        sb = ctx.enter_context(tc.tile_pool(name="sb", bufs=2))
        sb2 = ctx.enter_context(tc.tile_pool(name="sb2", bufs=2))
        sbg = ctx.enter_context(tc.tile_pool(name="sbg", bufs=2))
        psb = ctx.enter_context(tc.tile_pool(name="psb", bufs=2, space="PSUM"))
        pss = ctx.enter_context(tc.tile_pool(name="pss", bufs=2, space="PSUM"))
        pst = ctx.enter_context(tc.tile_pool(name="pst", bufs=2, space="PSUM"))

        io128 = consts.tile([128, 128], bf16)
        nc.sync.dma_start(out=io128[:], in_=inp["io128"][:])
        io64 = consts.tile([128, 64], bf16)
        nc.sync.dma_start(out=io64[:], in_=inp["io64"][:])
        ident = consts.tile([128, 128], bf16)
        nc.sync.dma_start(out=ident[:], in_=inp["ident"][:])

        # zero the never-written pad tail rows of hT (rows NB*BLK .. R)
        ntail = R - NB * BLK
        ztail = consts.tile([128, 512], bf16)
        nc.any.memset(ztail[:], 0.0)
        for l, w_ in ((1, 256), (2, 512)):
            nc.sync.dma_start(out=hT_d[l][NB * BLK:R, :],
                              in_=ztail[0:ntail, 0:w_])

        pool_sb = consts.tile([64, 257], f32)
        nc.any.memset(pool_sb[:], 0.0)

        # ---------------- layers ----------------
        # layer consts up front: phase A of layer l+1 is emitted in two bulk
        # chunks inside layer l's attention loop, so weights must be resident
        wsb_l, b2_l = {}, {}
        for li, (fin, H, C) in enumerate(DIMS, 1):
            HC = H * C
            nkc = max(1, fin // 128)
            KC = fin // nkc
            w = lay.tile([KC, nkc * (HC + 8)], bf16, tag=f"wsb{li}")
            for kc in range(nkc):
                nc.sync.dma_start(
                    out=w[:, kc * (HC + 8):(kc + 1) * (HC + 8)],
                    in_=inp[f"Wcat{li}"][kc * KC:(kc + 1) * KC, :])
            wsb_l[li] = w
            b2t = lay.tile([128, HC], bf16, tag=f"b2_{li}")
            nc.sync.dma_start(out=b2t[:], in_=inp[f"b2r{li}"][:])
            b2_l[li] = b2t

        def phase_a_iter(li, rt):
            fin, H, C = DIMS[li - 1]
            HC = H * C
            W2 = HC + 4
            nkc = max(1, fin // 128)
            KC = fin // nkc
            wsb = wsb_l[li]
            xe = xe_d[li - 1]
            pxs = psb.tile([128, HC], f32, tag="big")
            pal = pss.tile([128, 4 * T + 12], f32, tag="sm")
            for kc in range(nkc):
                ht = sb.tile([KC, 128], bf16, tag="ht", bufs=3)
                if li == 1:
                    nc.sync.dma_start(
                        out=ht[:],
                        in_=inp["xT"][:, rt * 128:(rt + 1) * 128])
                else:
                    hrm = sb.tile([128, KC], bf16, tag="hrm", bufs=3)
                    nc.sync.dma_start(
                        out=hrm[:],
                        in_=hT_d[li - 1][rt * 128:(rt + 1) * 128,
                                         kc * KC:(kc + 1) * KC])
                    htp = pst.tile([128, 128], bf16, tag="ps3b")
                    nc.tensor.transpose(htp[0:KC, :], hrm[:], ident[:])
                    nc.scalar.activation(out=ht[:], in_=htp[0:KC, :],
                                         func=AF.Copy)
                ws = wsb[:, kc * (HC + 8):(kc + 1) * (HC + 8)]
                nc.tensor.matmul(pxs[:], ht[:], ws[:, 0:HC],
                                 start=(kc == 0), stop=(kc == nkc - 1))
                nc.tensor.matmul(pal[:, 0:8], ht[:], ws[:, HC:HC + 8],
                                 start=(kc == 0), stop=(kc == nkc - 1))
            xs = sb.tile([128, W2], f8, tag="xs", bufs=3)
            nc.scalar.activation(out=xs[:, 0:HC], in_=pxs[:], func=AF.Copy)
            nc.scalar.activation(out=xs[:, HC:HC + 4], in_=pal[:, 0:4],
                                 func=AF.Copy)
            al4 = sb.tile([128, 4], bf16, tag="al4", bufs=3)
            nc.scalar.activation(out=al4[:], in_=pal[:, 4:8], func=AF.Copy)
            nc.scalar.dma_start(out=xe[rt * 128:(rt + 1) * 128, :],
                                in_=xs[:])
            nc.sync.dma_start(out=ald_d[li - 1][rt * 128:(rt + 1) * 128, :],
                              in_=al4[:])

        for rt in range(R // 128):
            phase_a_iter(1, rt)

        NRT = R // 128
        PA_SPLIT = 25          # rows 0..3199 stored once blocks 0..25 done

        for li, (fin, H, C) in enumerate(DIMS, 1):
            HC = H * C
            W2 = HC + 4
            xe = xe_d[li - 1]
            xf = xf_d[li - 1]
            b2 = b2_l[li]

            # ---- AllGather [xs | al_s] ----
            nc.gpsimd.collective_compute(
                "AllGather", AO.bypass, replica_groups=RG,
                ins=[xe[:]], outs=[xf[:]])

            # ---- attention + aggregation (fully unrolled for pipelining) ----
            # per-block record loads, prefetched two blocks ahead so they
            # never sit behind the hT stores in SP's queue
            recbufs = {}

            def load_rec(j):
                if j >= NB:
                    return
                TBj = Tb[j]
                irj = sb.tile([128, 2 * T + 1], i32, tag="ir", bufs=4)
                nc.sync.dma_start(out=irj[:],
                                  in_=inp["rec"][j * 128:j * 128 + 128,
                                                 0:2 * T + 1])
                alelj = sb.tile([128, 4 * T], i32, tag="alel", bufs=4)
                nc.sync.dma_start(
                    out=alelj[:, 0:4 * TBj],
                    in_=inp["rec"][j * 128:j * 128 + 128,
                                   2 * T + 1 + (li - 1) * 4 * T:
                                   2 * T + 1 + (li - 1) * 4 * T + 4 * TBj])
                ald4j = sb.tile([128, 4], bf16, tag="ald4", bufs=4)
                nc.sync.dma_start(out=ald4j[:],
                                  in_=ald_d[li - 1][j * BLK:j * BLK + 128, :])
                recbufs[j] = (irj, alelj, ald4j)

            load_rec(0)
            load_rec(1)
            for i in range(NB):
                TB = Tb[i]
                TH = (TB + 1) // 2            # first half-split of the tiles
                st128 = i * 128
                stblk = i * BLK
                load_rec(i + 2)
                ir, alel, ald4 = recbufs.pop(i)
                gat = sbg.tile([128, T * W2], f8, tag="gat", bufs=4)
                for t in range(TB):
                    nc.gpsimd.indirect_dma_start(
                        out=gat[:, t * W2:(t + 1) * W2], out_offset=None,
                        in_=xf[:],
                        in_offset=bass.IndirectOffsetOnAxis(
                            ap=ir[:, t:t + 1], axis=0))
                # one-hot [edge, dst] per tile (bf16, 4x tensor_scalar mode)
                # + transposed copy for the al_d one-hot expand on PE
                sall = sbg.tile([128, T * 128], bf16, tag="sall", bufs=3)
                dps = pss.tile([128, 4 * T + 12], f32, tag="sm")
                smp = pss.tile([128, 4 * T + 12], f32, tag="sm")
                wall = sb.tile([128, 4 * T], f32, tag="wall")
                wbf = sb.tile([128, 4 * T], bf16, tag="wbf")
                lk = sb.tile([128, 4 * T], f32, tag="lk")
                nps = psb.tile([128, HC], f32, tag="big")
                for t in range(TB):
                    nc.vector.tensor_scalar(
                        out=sall[:, t * 128:(t + 1) * 128], in0=io128[:],
                        scalar1=ir[:, T + t:T + t + 1].bitcast(f32),
                        scalar2=None, op0=AO.is_equal)
                    trp = pst.tile([128, 128], bf16, tag="ps3b")
                    nc.tensor.transpose(trp[:],
                                        sall[:, t * 128:(t + 1) * 128],
                                        ident[:])
                    sns = sb.tile([128, 128], bf16, tag="sns", bufs=4)
                    nc.scalar.activation(out=sns[:], in_=trp[:], func=AF.Copy)
                    nc.tensor.matmul(smp[:, 12 + 4 * t:16 + 4 * t], sns[:],
                                     ald4[:], start=True, stop=True)
                # logits (two tile-halves so val work starts at half-gather):
                # gathered al_s + expanded al_d + host al_e, leaky, exp
                for h0, h1 in ((0, TH), (TH, TB)):
                    hn = h1 - h0
                    nc.vector.tensor_tensor(
                        out=wall[:, 4 * h0:4 * h1]
                            .rearrange("p (t h) -> p t h", t=hn),
                        in0=gat[:, h0 * W2:h1 * W2]
                            .rearrange("p (t w) -> p t w", t=hn)[:, :, HC:HC + 4],
                        in1=smp[:, 12 + 4 * h0:12 + 4 * h1]
                            .rearrange("p (t h) -> p t h", t=hn),
                        op=AO.add)
                    nc.vector.tensor_tensor(
                        out=wall[:, 4 * h0:4 * h1],
                        in0=wall[:, 4 * h0:4 * h1],
                        in1=alel[:, 4 * h0:4 * h1].bitcast(f32), op=AO.add)
                    nc.vector.tensor_scalar(
                        out=lk[:, 4 * h0:4 * h1], in0=wall[:, 4 * h0:4 * h1],
                        scalar1=0.2, scalar2=None, op0=AO.mult)
                    nc.vector.tensor_tensor(
                        out=wall[:, 4 * h0:4 * h1], in0=wall[:, 4 * h0:4 * h1],
                        in1=lk[:, 4 * h0:4 * h1], op=AO.max)
                    nc.scalar.activation(out=wbf[:, 4 * h0:4 * h1],
                                         in_=wall[:, 4 * h0:4 * h1],
                                         func=AF.Exp)
                    if HC == 512 and h0 == TH:
                        wf32 = sb.tile([128, 8], f32, tag="wf32")
                        nc.scalar.activation(
                            out=wf32[:], in_=wall[:, 4 * TH:4 * TH + 8],
                            func=AF.Exp)
                    # weighting + aggregation for this half; on the wide
                    # layer spread the fp8 (1x-mode) multiplies across
                    # engines: Pool takes early-half tiles (weights ready
                    # before its gather burst ends), Act two strided tiles
                    for t in range(h0, h1):
                        val = sb.tile([128, HC], bf16, tag="val", bufs=4)
                        if HC == 512 and 4 <= t < min(9, TH):
                            veng = nc.gpsimd
                        else:
                            veng = nc.vector
                        if HC == 512 and TH <= t < TH + 2:
                            for h_ in range(H):
                                nc.scalar.activation(
                                    out=val[:].rearrange(
                                        "p (c h) -> p c h", h=H)[:, :, h_],
                                    in_=gat[:, t * W2:t * W2 + HC]
                                        .rearrange("p (c h) -> p c h",
                                                   h=H)[:, :, h_],
                                    func=AF.Copy,
                                    scale=wf32[:, (t - TH) * 4 + h_:
                                               (t - TH) * 4 + h_ + 1])
                        else:
                            veng.tensor_tensor(
                                out=val[:].rearrange("p (c h) -> p c h", h=H),
                                in0=gat[:, t * W2:t * W2 + HC]
                                    .rearrange("p (c h) -> p c h", h=H),
                                in1=wbf[:, t * 4:(t + 1) * 4].unsqueeze(1)
                                    .to_broadcast([128, C, 4]),
                                op=AO.mult)
                        nc.tensor.matmul(nps[:],
                                         sall[:, t * 128:(t + 1) * 128],
                                         val[:], start=(t == 0),
                                         stop=(t == TB - 1))
                        nc.tensor.matmul(dps[:, 0:4],
                                         sall[:, t * 128:(t + 1) * 128],
                                         wbf[:, t * 4:(t + 1) * 4],
                                         start=(t == 0), stop=(t == TB - 1))
                rden = sb.tile([128, 4], f32, tag="rden")
                nc.vector.tensor_scalar(out=rden[:], in0=dps[:, 0:4],
                                        scalar1=1e-30, scalar2=None,
                                        op0=AO.max)
                nc.vector.reciprocal(out=rden[:], in_=rden[:])
                hh = sb2.tile([128, HC + 1], bf16, tag="hh")
                for h in range(H):
                    nc.scalar.activation(
                        out=hh[:, 0:HC].rearrange("p (c h) -> p c h",
                                                  h=H)[:, :, h],
                        in_=nps[:].rearrange("p (c h) -> p c h", h=H)[:, :, h],
                        func=AF.Copy, scale=rden[:, h:h + 1])
                nc.vector.tensor_tensor(out=hh[:, 0:HC], in0=hh[:, 0:HC],
                                        in1=b2[:], op=AO.add)
                # ELU: (exp(min(h,0)) + relu(h)) - 1
                zn = sb2.tile([128, HC], bf16, tag="zn")
                nc.vector.tensor_scalar(out=zn[:], in0=hh[:, 0:HC],
                                        scalar1=0.0, scalar2=None, op0=AO.min)
                nc.scalar.activation(out=zn[:], in_=zn[:], func=AF.Exp)
                rl = sb2.tile([128, HC], bf16, tag="rl")
                nc.vector.tensor_scalar(out=rl[:], in0=hh[:, 0:HC],
                                        scalar1=0.0, scalar2=None, op0=AO.max)
                nc.vector.tensor_tensor(out=zn[:], in0=zn[:], in1=rl[:],
                                        op=AO.add)
                nc.vector.tensor_scalar(out=hh[:, 0:HC], in0=zn[:],
                                        scalar1=-1.0, scalar2=None, op0=AO.add)
                if li < 3:
                    nc.scalar.dma_start(
                        out=hT_d[li][stblk:stblk + BLK, :],
                        in_=hh[0:BLK, 0:HC])
                    if i == 25:
                        for rt in range(PA_SPLIT):
                            phase_a_iter(li + 1, rt)
                else:
                    nc.gpsimd.memset(hh[:, HC:HC + 1], 1.0)
                    bt = sb.tile([128, 64], bf16, tag="bt")
                    nc.vector.tensor_scalar(
                        out=bt[:], in0=io64[:],
                        scalar1=ir[:, 2 * T:2 * T + 1].bitcast(f32),
                        scalar2=None, op0=AO.is_equal)
                    pps = pst.tile([64, 257], f32, tag="ps3")
                    nc.tensor.matmul(pps[:], bt[:], hh[:],
                                     start=True, stop=True)
                    nc.vector.tensor_tensor(out=pool_sb[:], in0=pool_sb[:],
                                            in1=pps[:], op=AO.add)

            if li < 3:
                for rt in range(PA_SPLIT, NRT):
                    phase_a_iter(li + 1, rt)

        # ---------------- final MLP ----------------
        nc.sync.dma_start(out=pool_i[:], in_=pool_sb[:])
        nc.gpsimd.collective_compute("AllReduce", AO.add, replica_groups=RG,
                                     ins=[pool_i[:]], outs=[pool_o[:]])
        pool2 = sb.tile([64, 257], f32, tag="pool2")
        nc.sync.dma_start(out=pool2[:], in_=pool_o[:])
        cnt = sb.tile([64, 1], f32, tag="cnt")
        nc.vector.tensor_scalar(out=cnt[:], in0=pool2[:, 256:257], scalar1=1.0,
                                scalar2=None, op0=AO.max)
        nc.vector.reciprocal(out=cnt[:], in_=cnt[:])
        nc.vector.tensor_scalar(out=pool2[:, 0:256], in0=pool2[:, 0:256],
                                scalar1=cnt[:], scalar2=None, op0=AO.mult)
        identf = consts.tile([64, 64], f32)
        nc.sync.dma_start(out=identf[:], in_=inp["identf"][:])
        pts = sb.tile([128, 128], f32, tag="pts")
        for ch in range(2):
            ptp = pst.tile([128, 64], f32, tag="ps3")
            nc.tensor.transpose(ptp[:], pool2[:, ch * 128:(ch + 1) * 128],
                                identf[:])
            nc.vector.tensor_copy(out=pts[:, ch * 64:(ch + 1) * 64],
                                  in_=ptp[:])
        wf1 = sb.tile([128, 64], f32, tag="wf1")
        for ch in range(2):
            nc.sync.dma_start(out=wf1[:, ch * 32:(ch + 1) * 32],
                              in_=inp["Wf1"][ch * 128:(ch + 1) * 128, :])
        z1p = pst.tile([64, 32], f32, tag="ps3")
        for ch in range(2):
            nc.tensor.matmul(z1p[:], pts[:, ch * 64:(ch + 1) * 64],
                             wf1[:, ch * 32:(ch + 1) * 32],
                             start=(ch == 0), stop=(ch == 1))
        gf = sb.tile([64, 32], f32, tag="gf")
        nc.sync.dma_start(out=gf[:], in_=inp["gfr"][:])
        nc.vector.tensor_scalar(out=gf[:], in0=gf[:], scalar1=BNC,
                                scalar2=None, op0=AO.mult)
        b2f = sb.tile([64, 32], f32, tag="b2f")
        nc.sync.dma_start(out=b2f[:], in_=inp["bf1r"][:])
        nc.vector.tensor_tensor(out=b2f[:], in0=b2f[:], in1=gf[:], op=AO.mult)
        bbf = sb.tile([64, 32], f32, tag="bbf")
        nc.sync.dma_start(out=bbf[:], in_=inp["bbfr"][:])
        nc.vector.tensor_tensor(out=b2f[:], in0=b2f[:], in1=bbf[:], op=AO.add)
        zf = sb.tile([64, 32], f32, tag="zf")
        nc.vector.tensor_tensor(out=zf[:], in0=z1p[:], in1=gf[:], op=AO.mult)
        nc.vector.tensor_tensor(out=zf[:], in0=zf[:], in1=b2f[:], op=AO.add)
        zn2 = sb.tile([64, 32], f32, tag="zn2")
        nc.vector.tensor_scalar(out=zn2[:], in0=zf[:], scalar1=0.0,
                                scalar2=None, op0=AO.min)
        nc.scalar.activation(out=zn2[:], in_=zn2[:], func=AF.Exp)
        rl2 = sb.tile([64, 32], f32, tag="rl2")
        nc.scalar.activation(out=rl2[:], in_=zf[:], func=AF.Relu)
        nc.vector.scalar_tensor_tensor(out=zf[:], in0=zn2[:], scalar=-1.0,
                                       in1=rl2[:], op0=AO.add, op1=AO.add)
        ztp = pst.tile([32, 64], f32, tag="ps3")
        nc.tensor.transpose(ztp[:], zf[:], identf[:])
        zts = sb.tile([32, 64], f32, tag="zts")
        nc.vector.tensor_copy(out=zts[:], in_=ztp[:])
        wf2 = sb.tile([32, 2], f32, tag="wf2")
        nc.sync.dma_start(out=wf2[:], in_=inp["Wf2"][:])
        z2p = pst.tile([64, 2], f32, tag="ps3")
        nc.tensor.matmul(z2p[:], zts[:], wf2[:], start=True, stop=True)
        bf2 = sb.tile([64, 2], f32, tag="bf2")
        nc.sync.dma_start(out=bf2[:], in_=inp["bf2r"][:])
        z2 = sb.tile([64, 2], f32, tag="z2")
        nc.vector.tensor_tensor(out=z2[:], in0=z2p[:], in1=bf2[:], op=AO.add)
        mrow = sb.tile([64, 1], f32, tag="mrow")
        nc.vector.tensor_reduce(out=mrow[:], in_=z2[:],
                                axis=mybir.AxisListType.X, op=AO.max)
        nc.vector.tensor_scalar(out=z2[:], in0=z2[:], scalar1=mrow[:],
                                scalar2=None, op0=AO.subtract)
        ez = sb.tile([64, 2], f32, tag="ez")
        nc.scalar.activation(out=ez[:], in_=z2[:], func=AF.Exp)
        ssum = sb.tile([64, 1], f32, tag="ssum")
        nc.vector.tensor_reduce(out=ssum[:], in_=ez[:],
                                axis=mybir.AxisListType.X, op=AO.add)
        nc.scalar.activation(out=ssum[:], in_=ssum[:], func=AF.Ln)
        nc.vector.tensor_scalar(out=z2[:], in0=z2[:], scalar1=ssum[:],
                                scalar2=None, op0=AO.subtract)
        nc.sync.dma_start(out=out_d[:, :], in_=z2[:])

    nc.compile()
    return nc


# ---------------------------------------------------------------- entry point
def kernel(**inputs):
    _patch_walrus()
    in_maps, T = _prep(inputs)
    if T not in _CACHE:
        _CACHE[T] = _build(T)
    nc = _CACHE[T]
    from concourse.bass_utils import run_bass_kernel_spmd
    res = run_bass_kernel_spmd(nc, in_maps, list(range(D))).results
    return np.asarray(res[0]["out"], dtype=np.float32)


# revision 71
# speedup vs baseline: 1.0207x; 1.0207x over previous
"""Trainium2 Bass kernel for nn_GAT_n2v_mean (3-layer edge-featured GAT + mean-pool + MLP).

Strategy (hardcoded): partition edges by dst owner across 8 cores (6250 nodes
each, dst-sorted, 127-node blocks + trash slot, per-block exact tile counts).
Self-loops are appended as ordinary edges on the host (edge_attr = per-dst
mean, PyG fill_value='mean'). All input-only quantities (edge means, al_e
logit terms) are precomputed on the host. Per layer: phase A computes an fp8
node table [xs | al_s] plus a bf16 al_d table (BN scale folded into the
weights, al_s/al_d projections folded in as extra matmul columns), the fp8
table is AllGathered, then a fully-unrolled per-block attention loop issues
one indirect row-gather per 128-edge tile (Pool engine kept DMA-only),
builds per-tile one-hot matrices with 4x-mode tensor_scalar ops, expands
al_d[dst] per edge via PE-transposed one-hots and tiny matmuls, computes
softmax weights without any max shift (logits are bounded; the shift
cancels), and aggregates via PSUM matmuls. The channel layout is interleaved
(c-major, h-minor); logits/exp stay fp32->bf16. Cross-block pipelining
relies on multi-buffered tiles, record prefetch two blocks ahead, a widened
SWDGE descriptor ring, and row-major hidden-state stores (transposes moved
to phase A, off the per-block critical path).
"""

import numpy as np
import ml_dtypes

BF = ml_dtypes.bfloat16

_PATCHED = False


def _patch_walrus():
    """Enable per-partition vector dynamic offsets in walrus codegen
    (needed for the indirect row gathers; off by default in this path)."""
    global _PATCHED
    if _PATCHED:
        return
    import concourse.bass_utils as _bu
    _orig = _bu.run_command

    def _patched(argv, **kw):
        if any("codegen" in str(a) for a in argv):
            argv = list(argv)
            i = argv.index("-i")
            argv[i:i] = ["--dge-levels=vector_dynamic_offsets"]
        return _orig(argv, **kw)

    _bu.run_command = _patched
    _PATCHED = True


# ---------------------------------------------------------------- host config
N, E, G, D = 50000, 800000, 64, 8
NPD = N // D          # nodes per device
BLK = 127             # real node slots per block (slot 127 = trash)
NB = (NPD + BLK - 1) // BLK
R = NB * 128          # padded local rows in the node table
EPS = 1e-5
BNC = float(1.0 / np.sqrt(1.0 + EPS))
DIMS = [(32, 4, 64), (256, 4, 128), (512, 4, 64)]

_CACHE = {}


def _interleave_cols(H, C):
    """new col (c*H + h) <- old col (h*C + c); returns old index per new col."""
    newc = np.arange(H * C)
    c, h = newc // H, newc % H
    return h * C + c


def _prep(inputs):
    """Host-side sharding/layout prep (numpy only). Returns (in_maps, T)."""
    src_g = np.asarray(inputs["edge_index"][0], dtype=np.int64)
    dst_g = np.asarray(inputs["edge_index"][1], dtype=np.int64)
    ef = np.asarray(inputs["edge_feature"], dtype=np.float32)
    batch = np.asarray(inputs["batch"], dtype=np.int64)
    x = np.asarray(inputs["x"], dtype=np.float32)

    # self-loop edge features: per-dst mean of incoming edge features
    deg = np.bincount(dst_g, minlength=N).astype(np.float32)
    emean = np.zeros((N, 6), np.float32)
    np.add.at(emean, dst_g, ef)
    emean /= np.maximum(deg, 1.0)[:, None]

    src_a = np.concatenate([src_g, np.arange(N, dtype=np.int64)])
    dst_a = np.concatenate([dst_g, np.arange(N, dtype=np.int64)])
    ea_a = np.concatenate([ef, emean], axis=0)

    # per-layer host-computable logit projections Ae[f, h] and weight folds
    Ae_l, Wcat_l, b2_l = [], [], []
    prev_n2o = None
    for li, (fin, H, C) in enumerate(DIMS, 1):
        HC = H * C
        W = np.asarray(inputs[f"W{li}"], np.float32)
        a_s = np.asarray(inputs[f"as{li}"], np.float32)
        a_d = np.asarray(inputs[f"ad{li}"], np.float32)
        a_e = np.asarray(inputs[f"ae{li}"], np.float32)
        g_ = np.asarray(inputs[f"g{li}"], np.float32)
        b_ = np.asarray(inputs[f"b{li}"], np.float32)
        bb_ = np.asarray(inputs[f"bb{li}"], np.float32)
        We = np.asarray(inputs[f"We{li}"], np.float32)

        Ae = np.zeros((6, H), np.float32)
        WA = np.zeros((fin, 8), np.float32)
        for h in range(H):
            Ae[:, h] = We[:, h * C:(h + 1) * C] @ a_e[h]
            WA[:, h] = W[:, h * C:(h + 1) * C] @ a_s[h]
            WA[:, 4 + h] = W[:, h * C:(h + 1) * C] @ a_d[h]
        Ae_l.append(Ae)

        gc = g_ * BNC
        n2o = _interleave_cols(H, C)
        Wcat = np.concatenate([(W * gc[None, :])[:, n2o], WA], axis=1)
        if prev_n2o is not None:
            Wcat = Wcat[prev_n2o, :]
        Wcat_l.append(Wcat.astype(BF))
        b2_l.append((b_ * gc + bb_)[n2o].astype(BF))
        prev_n2o = n2o

    per_dev = []
    owner = dst_a // NPD
    for d in range(D):
        m = owner == d
        s, t = src_a[m], dst_a[m]
        ea = ea_a[m]
        loc = t - d * NPD
        blk = loc // BLK
        rel = loc % BLK
        order = np.argsort(blk, kind="stable")
        s, ea, blk, rel = s[order], ea[order], blk[order], rel[order]
        cnt = np.bincount(blk, minlength=NB)
        per_dev.append((s, ea, blk, rel, cnt))

    # dense tile-major slotting: slot k = (t = k // 128, p = k % 128)
    # per-block tile counts: max over devices (SPMD program is shared)
    cnts = np.stack([per_dev[d][4] for d in range(D)])      # [D, NB]
    Tb = tuple(int(t) for t in np.ceil(cnts.max(axis=0) / 128).astype(int))
    T = max(Tb)

    in_maps = []
    for d in range(D):
        s, ea, blk, rel, cnt = per_dev[d]
        idx_s = np.zeros((NB, T * 128), np.int32)     # global row in xf
        relm = np.full((NB, T * 128), 127.0, np.float32)
        alem = np.zeros((NB, T * 128, 12), np.float32)
        off = np.concatenate([[0], np.cumsum(cnt)])
        for bi in range(NB):
            e0, e1 = off[bi], off[bi + 1]
            k = e1 - e0
            ss = s[e0:e1]
            idx_s[bi, :k] = ((ss // NPD) * R + (ss % NPD)).astype(np.int32)
            relm[bi, :k] = rel[e0:e1].astype(np.float32)
            alem[bi, :k, :] = np.stack(
                [ea[e0:e1] @ Ae_l[li] for li in range(3)],
                axis=1).reshape(-1, 12)
        tp = lambda a: a.reshape(NB, T, 128).transpose(0, 2, 1)
        bb = np.full((NB, 128, 1), -1.0, np.float32)
        for bi in range(NB):
            lo = bi * BLK
            n = min(BLK, NPD - lo)
            if n > 0:
                bb[bi, :n, 0] = batch[d * NPD + lo: d * NPD + lo + n]
        # ale cols layer-major: l*4T + t*4 + h
        aleT = alem.reshape(NB, T, 128, 12).transpose(0, 2, 3, 1)
        aleT = aleT.reshape(NB, 128, 3, 4, T).transpose(0, 1, 2, 4, 3)
        aleT = aleT.reshape(NB, 128, 12 * T)
        rec = np.concatenate(
            [tp(idx_s), tp(relm.view(np.int32)), bb.view(np.int32),
             aleT.view(np.int32)], axis=2
        ).reshape(NB * 128, 14 * T + 1).copy()

        xT = np.zeros((32, R), BF)
        xT[:, :NPD] = x[d * NPD:(d + 1) * NPD].T.astype(BF)

        im = {
            "xT": xT,
            "rec": rec,
            "io128": np.broadcast_to(np.arange(128, dtype=np.float32),
                                     (128, 128)).astype(BF).copy(),
            "io64": np.broadcast_to(np.arange(64, dtype=np.float32),
                                    (128, 64)).astype(BF).copy(),
            "ident": np.eye(128, dtype=np.float32).astype(BF),
            "identf": np.eye(64, dtype=np.float32),
        }
        for li in range(1, 4):
            im[f"Wcat{li}"] = Wcat_l[li - 1]
            im[f"b2r{li}"] = np.broadcast_to(
                b2_l[li - 1], (128, len(b2_l[li - 1]))).copy()
        n2o3 = _interleave_cols(4, 64)
        im["Wf1"] = np.asarray(inputs["Wf1"], np.float32)[n2o3, :].copy()
        im["Wf2"] = np.asarray(inputs["Wf2"], np.float32)
        im["bf1r"] = np.broadcast_to(np.asarray(inputs["bf1"], np.float32),
                                     (64, 32)).copy()
        im["gfr"] = np.broadcast_to(np.asarray(inputs["gf"], np.float32),
                                    (64, 32)).copy()
        im["bbfr"] = np.broadcast_to(np.asarray(inputs["bbf"], np.float32),
                                     (64, 32)).copy()
        im["bf2r"] = np.broadcast_to(np.asarray(inputs["bf2"], np.float32),
                                     (64, 2)).copy()
        in_maps.append(im)
    return in_maps, (T, Tb)


def nk2(li):
    return DIMS[li - 1][1] * DIMS[li - 1][2] // 128


# ---------------------------------------------------------------- device prog
def _build(TT_):
    T, Tb = TT_
    import concourse.bass as bass
    import concourse.bacc as bacc
    import concourse.mybir as mybir
    import concourse.tile as tile
    from contextlib import ExitStack

    f32 = mybir.dt.float32
    bf16 = mybir.dt.bfloat16
    f8 = mybir.dt.float8e4
    i32 = mybir.dt.int32
    AO = mybir.AluOpType
    AF = mybir.ActivationFunctionType
    RG = [list(range(D))]
    RECW = 14 * T + 1

    nc = bacc.Bacc(None, target_bir_lowering=False, debug=True,
                   dynamic_dma_scratch_size=49152)

    # ---- I/O
    inp = {}
    def di(name, shape, dt=f32):
        inp[name] = nc.declare_dram_parameter(name, list(shape), dt,
                                              isOutput=False)
        return inp[name]

    di("xT", (32, R), bf16)
    di("rec", (NB * 128, RECW), i32)
    di("io128", (128, 128), bf16)
    di("io64", (128, 64), bf16)
    di("ident", (128, 128), bf16)
    di("identf", (64, 64))
    for li, (fin, H, C) in enumerate(DIMS, 1):
        HC = H * C
        di(f"Wcat{li}", (fin, HC + 8), bf16)
        di(f"b2r{li}", (128, HC), bf16)
    di("Wf1", (256, 32)); di("Wf2", (32, 2))
    di("bf1r", (64, 32)); di("gfr", (64, 32)); di("bbfr", (64, 32))
    di("bf2r", (64, 2))
    out_d = nc.declare_dram_parameter("out", [64, 2], f32, isOutput=True)

    # ---- internal DRAM
    Ws = [DIMS[i][1] * DIMS[i][2] for i in range(3)]
    xe_d = [nc.dram_tensor(f"xe{l}", [R, Ws[l - 1] + 4], f8)
            for l in (1, 2, 3)]
    xf_d = [nc.dram_tensor(f"xf{l}", [D * R, Ws[l - 1] + 4], f8,
                           addr_space="Shared") for l in (1, 2, 3)]
    ald_d = [nc.dram_tensor(f"ald{l}", [R, 4], bf16) for l in (1, 2, 3)]
    hT_d = [None,
            nc.dram_tensor("hT1", [R, 256], bf16),
            nc.dram_tensor("hT2", [R, 512], bf16)]
    pool_i = nc.dram_tensor("pool_i", [64, 257], f32)
    pool_o = nc.dram_tensor("pool_o", [64, 257], f32, addr_space="Shared")

    with ExitStack() as ctx:
        tc = ctx.enter_context(tile.TileContext(nc))
        consts = ctx.enter_context(tc.tile_pool(name="consts", bufs=1))
        lay = ctx.enter_context(tc.tile_pool(name="lay", bufs=1))
        sb = ctx.enter_context(tc.tile_pool(name="sb", bufs=2))
        sb2 = ctx.enter_context(tc.tile_pool(name="sb2", bufs=2))
        sbg = ctx.enter_context(tc.tile_pool(name="sbg", bufs=2))
        psb = ctx.enter_context(tc.tile_pool(name="psb", bufs=2, space="PSUM"))
        pss = ctx.enter_context(tc.tile_pool(name="pss", bufs=3, space="PSUM"))
        pst = ctx.enter_context(tc.tile_pool(name="pst", bufs=2, space="PSUM"))

        io128 = consts.tile([128, 128], bf16)
        nc.sync.dma_start(out=io128[:], in_=inp["io128"][:])
        io64 = consts.tile([128, 64], bf16)
        nc.sync.dma_start(out=io64[:], in_=inp["io64"][:])
        ident = consts.tile([128, 128], bf16)
        nc.sync.dma_start(out=ident[:], in_=inp["ident"][:])

        # zero the never-written pad tail rows of hT (rows NB*BLK .. R)
        ntail = R - NB * BLK
        ztail = consts.tile([128, 512], bf16)
        nc.any.memset(ztail[:], 0.0)
        for l, w_ in ((1, 256), (2, 512)):
            nc.sync.dma_start(out=hT_d[l][NB * BLK:R, :],
                              in_=ztail[0:ntail, 0:w_])

        pool_sb = consts.tile([64, 257], f32)
        nc.any.memset(pool_sb[:], 0.0)

        # ---------------- layers ----------------
        # layer consts up front: phase A of layer l+1 is emitted in two bulk
        # chunks inside layer l's attention loop, so weights must be resident
        wsb_l, b2_l = {}, {}
        for li, (fin, H, C) in enumerate(DIMS, 1):
            HC = H * C
            nkc = max(1, fin // 128)
            KC = fin // nkc
            w = lay.tile([KC, nkc * (HC + 8)], bf16, tag=f"wsb{li}")
            for kc in range(nkc):
                nc.sync.dma_start(
                    out=w[:, kc * (HC + 8):(kc + 1) * (HC + 8)],
                    in_=inp[f"Wcat{li}"][kc * KC:(kc + 1) * KC, :])
            wsb_l[li] = w
            b2t = lay.tile([128, HC], bf16, tag=f"b2_{li}")
            nc.sync.dma_start(out=b2t[:], in_=inp[f"b2r{li}"][:])
            b2_l[li] = b2t

        def phase_a_iter(li, rt):
            fin, H, C = DIMS[li - 1]
            HC = H * C
            W2 = HC + 4
            nkc = max(1, fin // 128)
            KC = fin // nkc
            wsb = wsb_l[li]
            xe = xe_d[li - 1]
            pxs = psb.tile([128, HC], f32, tag="big")
            pal = pss.tile([128, 4 * T + 12], f32, tag="sm")
            for kc in range(nkc):
                ht = sb.tile([KC, 128], bf16, tag="ht", bufs=3)
                if li == 1:
                    nc.sync.dma_start(
                        out=ht[:],
                        in_=inp["xT"][:, rt * 128:(rt + 1) * 128])
                else:
                    hrm = sb.tile([128, KC], bf16, tag="hrm", bufs=3)
                    nc.sync.dma_start(
                        out=hrm[:],
                        in_=hT_d[li - 1][rt * 128:(rt + 1) * 128,
                                         kc * KC:(kc + 1) * KC])
                    htp = pst.tile([128, 128], bf16, tag="ps3b")
                    nc.tensor.transpose(htp[0:KC, :], hrm[:], ident[:])
                    nc.scalar.activation(out=ht[:], in_=htp[0:KC, :],
                                         func=AF.Copy)
                ws = wsb[:, kc * (HC + 8):(kc + 1) * (HC + 8)]
                nc.tensor.matmul(pxs[:], ht[:], ws[:, 0:HC],
                                 start=(kc == 0), stop=(kc == nkc - 1))
                nc.tensor.matmul(pal[:, 0:8], ht[:], ws[:, HC:HC + 8],
                                 start=(kc == 0), stop=(kc == nkc - 1))
            xs = sb.tile([128, W2], f8, tag="xs", bufs=3)
            nc.scalar.activation(out=xs[:, 0:HC], in_=pxs[:], func=AF.Copy)
            nc.scalar.activation(out=xs[:, HC:HC + 4], in_=pal[:, 0:4],
                                 func=AF.Copy)
            al4 = sb.tile([128, 4], bf16, tag="al4", bufs=3)
            nc.scalar.activation(out=al4[:], in_=pal[:, 4:8], func=AF.Copy)
            nc.scalar.dma_start(out=xe[rt * 128:(rt + 1) * 128, :],
                                in_=xs[:])
            nc.sync.dma_start(out=ald_d[li - 1][rt * 128:(rt + 1) * 128, :],
                              in_=al4[:])

        for rt in range(R // 128):
            phase_a_iter(1, rt)

        NRT = R // 128
        PA_SPLIT = 25          # rows 0..3199 stored once blocks 0..25 done

        for li, (fin, H, C) in enumerate(DIMS, 1):
            HC = H * C
            W2 = HC + 4
            xe = xe_d[li - 1]
            xf = xf_d[li - 1]
            b2 = b2_l[li]

            # ---- AllGather [xs | al_s] ----
            nc.gpsimd.collective_compute(
                "AllGather", AO.bypass, replica_groups=RG,
                ins=[xe[:]], outs=[xf[:]])

            # ---- attention + aggregation (fully unrolled for pipelining) ----
            # per-block record loads, prefetched two blocks ahead so they
            # never sit behind the hT stores in SP's queue
            recbufs = {}

            def load_rec(j):
                if j >= NB:
                    return
                TBj = Tb[j]
                irj = sb.tile([128, 2 * T + 1], i32, tag="ir", bufs=4)
                nc.sync.dma_start(out=irj[:],
                                  in_=inp["rec"][j * 128:j * 128 + 128,
                                                 0:2 * T + 1])
                alelj = sb.tile([128, 4 * T], i32, tag="alel", bufs=4)
                nc.sync.dma_start(
                    out=alelj[:, 0:4 * TBj],
                    in_=inp["rec"][j * 128:j * 128 + 128,
                                   2 * T + 1 + (li - 1) * 4 * T:
                                   2 * T + 1 + (li - 1) * 4 * T + 4 * TBj])
                ald4j = sb.tile([128, 4], bf16, tag="ald4", bufs=4)
                nc.sync.dma_start(out=ald4j[:],
                                  in_=ald_d[li - 1][j * BLK:j * BLK + 128, :])
                recbufs[j] = (irj, alelj, ald4j)

            load_rec(0)
            load_rec(1)
            for i in range(NB):
                TB = Tb[i]
                TH = (TB + 1) // 2            # first half-split of the tiles
                st128 = i * 128
                stblk = i * BLK
                load_rec(i + 2)
                ir, alel, ald4 = recbufs.pop(i)
                gat = sbg.tile([128, T * W2], f8, tag="gat", bufs=4)
                for t in range(TB):
                    nc.gpsimd.indirect_dma_start(
                        out=gat[:, t * W2:(t + 1) * W2], out_offset=None,
                        in_=xf[:],
                        in_offset=bass.IndirectOffsetOnAxis(
                            ap=ir[:, t:t + 1], axis=0))
                # one-hot [edge, dst] per tile (bf16, 4x tensor_scalar mode)
                # + transposed copy for the al_d one-hot expand on PE
                sall = sbg.tile([128, T * 128], bf16, tag="sall", bufs=4)
                dps = pss.tile([128, 4 * T + 12], f32, tag="sm")
                smp = pss.tile([128, 4 * T + 12], f32, tag="sm")
                wall = sb.tile([128, 4 * T], f32, tag="wall")
                wbf = sb.tile([128, 4 * T], bf16, tag="wbf")
                lk = sb.tile([128, 4 * T], f32, tag="lk")
                nps = psb.tile([128, HC], f32, tag="big")
                for t in range(TB):
                    nc.vector.tensor_scalar(
                        out=sall[:, t * 128:(t + 1) * 128], in0=io128[:],
                        scalar1=ir[:, T + t:T + t + 1].bitcast(f32),
                        scalar2=None, op0=AO.is_equal)
                    trp = pst.tile([128, 128], bf16, tag="ps3b")
                    nc.tensor.transpose(trp[:],
                                        sall[:, t * 128:(t + 1) * 128],
                                        ident[:])
                    sns = sb.tile([128, 128], bf16, tag="sns", bufs=6)
                    nc.scalar.activation(out=sns[:], in_=trp[:], func=AF.Copy)
                    nc.tensor.matmul(smp[:, 12 + 4 * t:16 + 4 * t], sns[:],
                                     ald4[:], start=True, stop=True)
                # logits (two tile-halves so val work starts at half-gather):
                # gathered al_s + expanded al_d + host al_e, leaky, exp
                for h0, h1 in ((0, TH), (TH, TB)):
                    hn = h1 - h0
                    nc.vector.tensor_tensor(
                        out=wall[:, 4 * h0:4 * h1]
                            .rearrange("p (t h) -> p t h", t=hn),
                        in0=gat[:, h0 * W2:h1 * W2]
                            .rearrange("p (t w) -> p t w", t=hn)[:, :, HC:HC + 4],
                        in1=smp[:, 12 + 4 * h0:12 + 4 * h1]
                            .rearrange("p (t h) -> p t h", t=hn),
                        op=AO.add)
                    nc.vector.tensor_tensor(
                        out=wall[:, 4 * h0:4 * h1],
                        in0=wall[:, 4 * h0:4 * h1],
                        in1=alel[:, 4 * h0:4 * h1].bitcast(f32), op=AO.add)
                    nc.vector.tensor_scalar(
                        out=lk[:, 4 * h0:4 * h1], in0=wall[:, 4 * h0:4 * h1],
                        scalar1=0.2, scalar2=None, op0=AO.mult)
                    nc.vector.tensor_tensor(
                        out=wall[:, 4 * h0:4 * h1], in0=wall[:, 4 * h0:4 * h1],
                        in1=lk[:, 4 * h0:4 * h1], op=AO.max)
                    nc.scalar.activation(out=wbf[:, 4 * h0:4 * h1],
                                         in_=wall[:, 4 * h0:4 * h1],
                                         func=AF.Exp)
                    if HC == 512 and h0 == TH:
                        wf32 = sb.tile([128, 8], f32, tag="wf32")
                        nc.scalar.activation(
                            out=wf32[:], in_=wall[:, 4 * TH:4 * TH + 8],
                            func=AF.Exp)
                    # weighting + aggregation for this half; on the wide
                    # layer spread the fp8 (1x-mode) multiplies across
                    # engines: Pool takes early-half tiles (weights ready
                    # before its gather burst ends), Act two strided tiles
                    for t in range(h0, h1):
                        val = sb.tile([128, HC], bf16, tag="val", bufs=4)
                        if HC == 512 and 4 <= t < min(9, TH):
                            veng = nc.gpsimd
                        else:
                            veng = nc.vector
                        if HC == 512 and TH <= t < TH + 2:
                            for h_ in range(H):
                                nc.scalar.activation(
                                    out=val[:].rearrange(
                                        "p (c h) -> p c h", h=H)[:, :, h_],
                                    in_=gat[:, t * W2:t * W2 + HC]
                                        .rearrange("p (c h) -> p c h",
                                                   h=H)[:, :, h_],
                                    func=AF.Copy,
                                    scale=wf32[:, (t - TH) * 4 + h_:
                                               (t - TH) * 4 + h_ + 1])
                        else:
                            veng.tensor_tensor(
                                out=val[:].rearrange("p (c h) -> p c h", h=H),
                                in0=gat[:, t * W2:t * W2 + HC]
                                    .rearrange("p (c h) -> p c h", h=H),
                                in1=wbf[:, t * 4:(t + 1) * 4].unsqueeze(1)
                                    .to_broadcast([128, C, 4]),
                                op=AO.mult)
                        nc.tensor.matmul(nps[:],
                                         sall[:, t * 128:(t + 1) * 128],
                                         val[:], start=(t == 0),
                                         stop=(t == TB - 1))
                        nc.tensor.matmul(dps[:, 0:4],
                                         sall[:, t * 128:(t + 1) * 128],
                                         wbf[:, t * 4:(t + 1) * 4],
                                         start=(t == 0), stop=(t == TB - 1))
                rden = sb.tile([128, 4], f32, tag="rden")
                nc.vector.tensor_scalar(out=rden[:], in0=dps[:, 0:4],
                                        scalar1=1e-30, scalar2=None,
                                        op0=AO.max)
                nc.vector.reciprocal(out=rden[:], in_=rden[:])
                hh = sb2.tile([128, HC + 1], bf16, tag="hh")
                for h in range(H):
                    nc.scalar.activation(
                        out=hh[:, 0:HC].rearrange("p (c h) -> p c h",
                                                  h=H)[:, :, h],
                        in_=nps[:].rearrange("p (c h) -> p c h", h=H)[:, :, h],
                        func=AF.Copy, scale=rden[:, h:h + 1])
                nc.vector.tensor_tensor(out=hh[:, 0:HC], in0=hh[:, 0:HC],
                                        in1=b2[:], op=AO.add)
                # ELU: (exp(min(h,0)) + relu(h)) - 1
                zn = sb2.tile([128, HC], bf16, tag="zn")
                nc.vector.tensor_scalar(out=zn[:], in0=hh[:, 0:HC],
                                        scalar1=0.0, scalar2=None, op0=AO.min)
                nc.scalar.activation(out=zn[:], in_=zn[:], func=AF.Exp)
                rl = sb2.tile([128, HC], bf16, tag="rl")
                nc.vector.tensor_scalar(out=rl[:], in0=hh[:, 0:HC],
                                        scalar1=0.0, scalar2=None, op0=AO.max)
                nc.vector.tensor_tensor(out=zn[:], in0=zn[:], in1=rl[:],
                                        op=AO.add)
                nc.vector.tensor_scalar(out=hh[:, 0:HC], in0=zn[:],
                                        scalar1=-1.0, scalar2=None, op0=AO.add)
                if li < 3:
                    nc.scalar.dma_start(
                        out=hT_d[li][stblk:stblk + BLK, :],
                        in_=hh[0:BLK, 0:HC])
                    if i == 25:
                        for rt in range(PA_SPLIT):
                            phase_a_iter(li + 1, rt)
                else:
                    nc.gpsimd.memset(hh[:, HC:HC + 1], 1.0)
                    bt = sb.tile([128, 64], bf16, tag="bt")
                    nc.vector.tensor_scalar(
                        out=bt[:], in0=io64[:],
                        scalar1=ir[:, 2 * T:2 * T + 1].bitcast(f32),
                        scalar2=None, op0=AO.is_equal)
                    pps = pst.tile([64, 257], f32, tag="ps3", bufs=1)
                    nc.tensor.matmul(pps[:], bt[:], hh[:],
                                     start=True, stop=True)
                    nc.vector.tensor_tensor(out=pool_sb[:], in0=pool_sb[:],
                                            in1=pps[:], op=AO.add)

            if li < 3:
                for rt in range(PA_SPLIT, NRT):
                    phase_a_iter(li + 1, rt)

        # ---------------- final MLP ----------------
        nc.sync.dma_start(out=pool_i[:], in_=pool_sb[:])
        nc.gpsimd.collective_compute("AllReduce", AO.add, replica_groups=RG,
                                     ins=[pool_i[:]], outs=[pool_o[:]])
        pool2 = sb.tile([64, 257], f32, tag="pool2")
        nc.sync.dma_start(out=pool2[:], in_=pool_o[:])
        cnt = sb.tile([64, 1], f32, tag="cnt")
        nc.vector.tensor_scalar(out=cnt[:], in0=pool2[:, 256:257], scalar1=1.0,
                                scalar2=None, op0=AO.max)
        nc.vector.reciprocal(out=cnt[:], in_=cnt[:])
        nc.vector.tensor_scalar(out=pool2[:, 0:256], in0=pool2[:, 0:256],
                                scalar1=cnt[:], scalar2=None, op0=AO.mult)
        identf = consts.tile([64, 64], f32)
        nc.sync.dma_start(out=identf[:], in_=inp["identf"][:])
        pts = sb.tile([128, 128], f32, tag="pts")
        for ch in range(2):
            ptp = pst.tile([128, 64], f32, tag="ps3", bufs=1)
            nc.tensor.transpose(ptp[:], pool2[:, ch * 128:(ch + 1) * 128],
                                identf[:])
            nc.vector.tensor_copy(out=pts[:, ch * 64:(ch + 1) * 64],
                                  in_=ptp[:])
        wf1 = sb.tile([128, 64], f32, tag="wf1")
        for ch in range(2):
            nc.sync.dma_start(out=wf1[:, ch * 32:(ch + 1) * 32],
                              in_=inp["Wf1"][ch * 128:(ch + 1) * 128, :])
        z1p = pst.tile([64, 32], f32, tag="ps3", bufs=1)
        for ch in range(2):
            nc.tensor.matmul(z1p[:], pts[:, ch * 64:(ch + 1) * 64],
                             wf1[:, ch * 32:(ch + 1) * 32],
                             start=(ch == 0), stop=(ch == 1))
        gf = sb.tile([64, 32], f32, tag="gf")
        nc.sync.dma_start(out=gf[:], in_=inp["gfr"][:])
        nc.vector.tensor_scalar(out=gf[:], in0=gf[:], scalar1=BNC,
                                scalar2=None, op0=AO.mult)
        b2f = sb.tile([64, 32], f32, tag="b2f")
        nc.sync.dma_start(out=b2f[:], in_=inp["bf1r"][:])
        nc.vector.tensor_tensor(out=b2f[:], in0=b2f[:], in1=gf[:], op=AO.mult)
        bbf = sb.tile([64, 32], f32, tag="bbf")
        nc.sync.dma_start(out=bbf[:], in_=inp["bbfr"][:])
        nc.vector.tensor_tensor(out=b2f[:], in0=b2f[:], in1=bbf[:], op=AO.add)
        zf = sb.tile([64, 32], f32, tag="zf")
        nc.vector.tensor_tensor(out=zf[:], in0=z1p[:], in1=gf[:], op=AO.mult)
        nc.vector.tensor_tensor(out=zf[:], in0=zf[:], in1=b2f[:], op=AO.add)
        zn2 = sb.tile([64, 32], f32, tag="zn2")
        nc.vector.tensor_scalar(out=zn2[:], in0=zf[:], scalar1=0.0,
                                scalar2=None, op0=AO.min)
        nc.scalar.activation(out=zn2[:], in_=zn2[:], func=AF.Exp)
        rl2 = sb.tile([64, 32], f32, tag="rl2")
        nc.scalar.activation(out=rl2[:], in_=zf[:], func=AF.Relu)
        nc.vector.scalar_tensor_tensor(out=zf[:], in0=zn2[:], scalar=-1.0,
                                       in1=rl2[:], op0=AO.add, op1=AO.add)
        ztp = pst.tile([32, 64], f32, tag="ps3", bufs=1)
        nc.tensor.transpose(ztp[:], zf[:], identf[:])
        zts = sb.tile([32, 64], f32, tag="zts")
        nc.vector.tensor_copy(out=zts[:], in_=ztp[:])
        wf2 = sb.tile([32, 2], f32, tag="wf2")
        nc.sync.dma_start(out=wf2[:], in_=inp["Wf2"][:])
        z2p = pst.tile([64, 2], f32, tag="ps3", bufs=1)
        nc.tensor.matmul(z2p[:], zts[:], wf2[:], start=True, stop=True)
        bf2 = sb.tile([64, 2], f32, tag="bf2")
        nc.sync.dma_start(out=bf2[:], in_=inp["bf2r"][:])
        z2 = sb.tile([64, 2], f32, tag="z2")
        nc.vector.tensor_tensor(out=z2[:], in0=z2p[:], in1=bf2[:], op=AO.add)
        mrow = sb.tile([64, 1], f32, tag="mrow")
        nc.vector.tensor_reduce(out=mrow[:], in_=z2[:],
                                axis=mybir.AxisListType.X, op=AO.max)
        nc.vector.tensor_scalar(out=z2[:], in0=z2[:], scalar1=mrow[:],
                                scalar2=None, op0=AO.subtract)
        ez = sb.tile([64, 2], f32, tag="ez")
        nc.scalar.activation(out=ez[:], in_=z2[:], func=AF.Exp)
        ssum = sb.tile([64, 1], f32, tag="ssum")
        nc.vector.tensor_reduce(out=ssum[:], in_=ez[:],
                                axis=mybir.AxisListType.X, op=AO.add)
        nc.scalar.activation(out=ssum[:], in_=ssum[:], func=AF.Ln)
        nc.vector.tensor_scalar(out=z2[:], in0=z2[:], scalar1=ssum[:],
                                scalar2=None, op0=AO.subtract)
        nc.sync.dma_start(out=out_d[:, :], in_=z2[:])

    nc.compile()
    return nc


# ---------------------------------------------------------------- entry point
def kernel(**inputs):
    _patch_walrus()
    in_maps, T = _prep(inputs)
    if T not in _CACHE:
        _CACHE[T] = _build(T)
    nc = _CACHE[T]
    from concourse.bass_utils import run_bass_kernel_spmd
    res = run_bass_kernel_spmd(nc, in_maps, list(range(D))).results
    return np.asarray(res[0]["out"], dtype=np.float32)
